# revision 1
# baseline (speedup 1.0000x reference)
"""Trainium2 Bass kernel for per-view cross-attention.

Reference computation (per view v of 1024, S=64 samples, D=256):
  qp = q @ Wq.T + pe ; kp = k @ Wk.T + pe ; vp = v @ Wv.T + pe
  attn = softmax(qp @ kp.T / sqrt(D))
  x = gelu(attn @ vp @ Wo.T + bo) + q
Sharding: data-parallel over the 1024 views across 8 cores (128 views each).

On-chip layout strategy: everything is kept in "transposed" space [D, rows]
(rows = view*64+s) so that the contraction dim D lands on SBUF partitions
without any on-chip input transposes. The host pre-transposes q/k/v shards to
[D, rows] (free: numpy) and post-transposes the [D, rows] output back.
v additionally needs its projected form in natural [row, D] layout for the
attn@v matmul; that drops out naturally by using vT as the matmul stationary.
"""

import sys
import os

for p in ("/opt/trn_rl_repo",):
    if p not in sys.path and os.path.isdir(p):
        sys.path.insert(0, p)

import numpy as np

V, S, D = 1024, 64, 256
N_CORES = 8
VC = V // N_CORES          # views per core
ROWS = VC * S              # 8192 rows per core
R = 512                    # rows per supertile (8 views)
NST = ROWS // R            # supertiles per core
NV = R // S                # views per supertile
GELU_GROUP = 4             # supertiles per gelu flush (ACT table amortization)
PROJ_BUFS = 3
SM_BUFS = 3
PS_S_BUFS = 1
PS_T_BUFS = 1
PS_A_BUFS = 3
PS_B_BUFS = 3
LD_BUFS = 3
SCALE = 1.0 / np.sqrt(np.float32(D)).astype(np.float32)

_CACHE = {}


def _make_posenc(d_hid, n_samples):
    pos = np.arange(n_samples, dtype=np.float64)[:, None]
    j = np.arange(d_hid)[None, :]
    angle = pos / np.power(10000.0, 2.0 * (j // 2) / d_hid)
    table = np.where(j % 2 == 0, np.sin(angle), np.cos(angle))
    return table.astype(np.float32)  # [S, D]


def _build(rows=ROWS, stage=99):
    import concourse.bass as bass
    import concourse.mybir as mybir
    import concourse.tile as tile
    from concourse.tile import add_dep_helper
    from concourse import bacc
    from contextlib import ExitStack

    fp32 = mybir.dt.float32
    f32r = mybir.dt.float32r
    bf16 = mybir.dt.bfloat16
    AF = mybir.ActivationFunctionType
    ALU = mybir.AluOpType
    n_st = rows // R

    nc = bacc.Bacc(None, target_bir_lowering=False)

    qT_d = nc.dram_tensor("qT", [D, rows], f32r, kind="ExternalInput")
    kT_d = nc.dram_tensor("kT", [D, rows], f32r, kind="ExternalInput")
    vT_d = nc.dram_tensor("vT", [D, rows], f32r, kind="ExternalInput")
    wq_d = nc.dram_tensor("WqT", [D, D], f32r, kind="ExternalInput")
    wk_d = nc.dram_tensor("WkT", [D, D], f32r, kind="ExternalInput")
    wv_d = nc.dram_tensor("WvT", [D, D], f32r, kind="ExternalInput")
    wo_d = nc.dram_tensor("WoT", [D, D], f32r, kind="ExternalInput")
    bo_d = nc.dram_tensor("bo", [D], fp32, kind="ExternalInput")
    pet_d = nc.dram_tensor("peT_rep", [D, R], fp32, kind="ExternalInput")
    pe_d = nc.dram_tensor("pe_nat", [S, D], f32r, kind="ExternalInput")
    e2_d = nc.dram_tensor("E2", [S, 128], f32r, kind="ExternalInput")
    id_d = nc.dram_tensor("I128", [128, 128], fp32, kind="ExternalInput")
    out_d = nc.dram_tensor("outT", [D, rows], fp32, kind="ExternalOutput")

    def r3(ap):  # [D, X] dram -> [128, 2, X] partition view
        return ap.rearrange("(kc p) r -> p kc r", p=128)

    with tile.TileContext(nc) as tc, ExitStack() as ctx:
        const = ctx.enter_context(tc.tile_pool(name="const", bufs=1))
        ld = ctx.enter_context(tc.tile_pool(name="ld", bufs=LD_BUFS))
        proj = ctx.enter_context(tc.tile_pool(name="proj", bufs=PROJ_BUFS))
        sm = ctx.enter_context(tc.tile_pool(name="sm", bufs=SM_BUFS))
        psA = ctx.enter_context(tc.tile_pool(name="psA", bufs=PS_A_BUFS, space="PSUM"))
        psB = ctx.enter_context(tc.tile_pool(name="psB", bufs=PS_B_BUFS, space="PSUM"))
        psS = ctx.enter_context(tc.tile_pool(name="psS", bufs=PS_S_BUFS, space="PSUM"))
        psT = ctx.enter_context(tc.tile_pool(name="psT", bufs=PS_T_BUFS, space="PSUM"))
        stg = ctx.enter_context(tc.tile_pool(name="stg", bufs=GELU_GROUP + 1))

        wq = const.tile([128, 2, D], f32r)
        wk = const.tile([128, 2, D], f32r)
        wv = const.tile([128, 2, D], f32r)
        wo = const.tile([128, 2, D], f32r)
        nc.sync.dma_start(wq, r3(wq_d[:]))
        nc.sync.dma_start(wk, r3(wk_d[:]))
        nc.sync.dma_start(wv, r3(wv_d[:]))
        nc.sync.dma_start(wo, r3(wo_d[:]))
        pet = const.tile([128, 2, R], fp32)
        nc.sync.dma_start(pet, r3(pet_d[:]))
        pe_sb = const.tile([S, D], f32r)
        nc.sync.dma_start(pe_sb, pe_d[:])
        e2 = const.tile([S, 128], f32r)
        nc.sync.dma_start(e2, e2_d[:])
        i128 = const.tile([128, 128], fp32)
        nc.sync.dma_start(i128, id_d[:])
        bo_sb = const.tile([128, 2], fp32)
        nc.sync.dma_start(bo_sb, bo_d.rearrange("(kc p) -> p kc", p=128))

        pending = []
        last_gelu = None
        last_exp = None
        for st in range(n_st):
            rs = slice(st * R, (st + 1) * R)
            qt = ld.tile([128, 2, R], f32r, tag="qt", bufs=GELU_GROUP + 2)
            kt = ld.tile([128, 2, R], f32r, tag="kt")
            vt = ld.tile([128, 2, R], f32r, tag="vt")
            nc.sync.dma_start(qt, r3(qT_d[:])[:, :, rs])
            nc.sync.dma_start(kt, r3(kT_d[:])[:, :, rs])
            nc.sync.dma_start(vt, r3(vT_d[:])[:, :, rs])

            # ---- projections into transposed space: xpT[dout, row] ----
            qpT = proj.tile([128, 2, R], fp32, tag="qpT")
            kpT = proj.tile([128, 2, R], fp32, tag="kpT")
            for w_sb, x_sb, o_sb in ((wq, qt, qpT), (wk, kt, kpT)):
                for mc in range(2):
                    ps = psA.tile([128, R], fp32, tag="psA", name="ps_proj")
                    for kc in range(2):
                        nc.tensor.matmul(
                            ps,
                            w_sb[:, kc, mc * 128:(mc + 1) * 128],
                            x_sb[:, kc, :],
                            start=(kc == 0),
                            stop=(kc == 1),
                        )
                    # evacuate PSUM fused with positional-encoding add
                    nc.vector.tensor_add(
                        out=o_sb[:, mc, :], in0=ps, in1=pet[:, mc, :]
                    )

            if stage <= 1:
                nc.sync.dma_start(r3(out_d[:])[:, :, rs], qpT)
                continue
            # ---- vp in natural [row, dout] layout (vT as stationary) ----
            vp = proj.tile([128, 4, D], fp32, tag="vp")
            for g in range(4):
                psv = psB.tile([128, D], fp32, tag="psB", name="ps_vp")
                for kc in range(2):
                    nc.tensor.matmul(
                        psv,
                        vt[:, kc, g * 128:(g + 1) * 128],
                        wv[:, kc, :],
                        start=(kc == 0),
                        stop=False,
                    )
                # pe add folded in as a matmul: E2.T @ pe = pe tiled over rows
                nc.tensor.matmul(psv, e2, pe_sb, start=False, stop=True)
                nc.scalar.copy(out=vp[:, g, :], in_=psv)

            if stage <= 2:
                nc.sync.dma_start(r3(out_d[:])[:, :, rs], vp.rearrange("p a b -> p (a b)")[:, None, :].rearrange("p o (a b) -> p (o a) b", a=2))
                continue
            # ---- scores: per view [64,64], packed [128(2 views), 4, 64] ----
            scps = psS.tile([128, 4, S], fp32, tag="scores")
            for v in range(NV):
                g, h = v // 2, v % 2
                for dc in range(2):
                    nc.tensor.matmul(
                        scps[h * 64:(h + 1) * 64, g, :],
                        qpT[:, dc, v * S:(v + 1) * S],
                        kpT[:, dc, v * S:(v + 1) * S],
                        start=(dc == 0),
                        stop=(dc == 1),
                        tile_position=(0, h * 64),
                    )

            # ---- softmax along free axis (no max-subtraction: |scores/16|<~10) ----
            attn = sm.tile([128, 4, S], fp32, tag="attn")
            _e = nc.scalar.activation(attn, scps, AF.Exp, scale=float(SCALE))
            # keep Exp-set ops contiguous on ACT: exp of a new gelu-group must
            # come after the previous group's last gelu
            if last_gelu is not None:
                add_dep_helper(_e.ins, last_gelu, sync=False,
                               reason="act-table grouping: exp after prior gelus")
            last_exp = _e.ins
            sums = sm.tile([128, 4], fp32, tag="sums")
            nc.vector.tensor_reduce(out=sums, in_=attn, axis=mybir.AxisListType.X, op=ALU.add)
            rec = sm.tile([128, 4], fp32, tag="rec")
            nc.vector.reciprocal(rec, sums)
            nc.vector.tensor_tensor(
                attn, attn, rec[:, :, None].to_broadcast((128, 4, S)), ALU.mult
            )

            if stage <= 3:
                nc.sync.dma_start(r3(out_d[:])[:, 0, st * R: st * R + 256], attn.rearrange("p a b -> p (a b)"))
                continue
            # ---- transpose attn packs; duplicate into both partition halves ----
            atps = psT.tile([128, 4, 128], fp32, tag="attnT")
            for g in range(4):
                for h in range(2):
                    nc.tensor.matmul(
                        atps[h * 64:(h + 1) * 64, g, :],
                        attn[:, g, :],
                        i128,
                        start=True,
                        stop=True,
                        tile_position=(0, h * 64),
                    )
            attnT = sm.tile([128, 4, 128], fp32, tag="attnT_sb")
            nc.scalar.copy(out=attnT, in_=atps)

            if stage <= 4:
                nc.sync.dma_start(r3(out_d[:])[:, 0, st * R: st * R + 512], attnT.rearrange("p a b -> p (a b)"))
                continue
            # ---- attn @ vp, directly in transposed space outT[d, row] ----
            # Concurrent row-group matmuls must not drain into the same
            # (partition, bank) pair: one PSUM tile per row-half h.
            outT = proj.tile([128, 2, R], f32r, tag="outT")
            for c in range(2):
                for h in range(2):
                    pso = psB.tile([128, 4, S], fp32, tag="psB", name="ps_av")
                    for g in range(4):
                        nc.tensor.matmul(
                            pso[:, g, :],
                            vp[h * 64:(h + 1) * 64, g, c * 128:(c + 1) * 128],
                            attnT[h * 64:(h + 1) * 64, g, h * 64:(h + 1) * 64],
                            start=True,
                            stop=True,
                            tile_position=(h * 64, 0),
                        )
                    # view v=2g+h lives at free offset v*64 of outT chunk c
                    o_ap = outT[:, c, :].rearrange(
                        "p (g two s) -> p g two s", two=2, s=S
                    )[:, :, h, :]
                    if c == 0:
                        nc.vector.tensor_copy(o_ap, pso)
                    else:
                        nc.scalar.copy(out=o_ap, in_=pso)

            if stage <= 5:
                nc.sync.dma_start(r3(out_d[:])[:, :, rs], outT)
                continue
            # ---- final projection, staged pre-gelu (Exp and Gelu live in
            # different ACT table sets; group gelus to amortize ~2.7us
            # table switches) ----
            pre = stg.tile([128, 2, R], fp32, tag="pre")
            for mc in range(2):
                psf = psA.tile([128, R], fp32, tag="psA", name="ps_fin")
                for kc in range(2):
                    nc.tensor.matmul(
                        psf,
                        wo[:, kc, mc * 128:(mc + 1) * 128],
                        outT[:, kc, :],
                        start=(kc == 0),
                        stop=(kc == 1),
                    )
                if mc == 0:
                    nc.vector.tensor_copy(pre[:, mc, :], psf)
                else:
                    nc.scalar.copy(out=pre[:, mc, :], in_=psf)
            pending.append((st, pre, qt))

            if len(pending) == GELU_GROUP or st == n_st - 1:
                for pst, ppre, pqt in pending:
                    outsb = proj.tile([128, 2, R], fp32, tag="outsb")
                    for mc in range(2):
                        _g = nc.scalar.activation(
                            out=outsb[:, mc, :], in_=ppre[:, mc, :],
                            func=AF.Gelu, bias=bo_sb[:, mc:mc + 1], scale=1.0,
                        )
                        if last_exp is not None:
                            add_dep_helper(_g.ins, last_exp, sync=False,
                                           reason="act-table grouping: gelu after group exps")
                        last_gelu = _g.ins
                        nc.vector.tensor_add(
                            out=outsb[:, mc, :], in0=outsb[:, mc, :],
                            in1=pqt[:, mc, :],
                        )
                    nc.sync.dma_start(
                        r3(out_d[:])[:, :, pst * R:(pst + 1) * R], outsb
                    )
                pending = []

    nc.finalize()
    return nc


def _get_nc():
    if "nc" not in _CACHE:
        _CACHE["nc"] = _build()
    return _CACHE["nc"]


def _host_inputs(q, k, v, Wq, Wk, Wv, Wo, bo):
    pe = _make_posenc(D, S)                      # [S, D]
    peT_rep = np.ascontiguousarray(np.tile(pe.T, (1, NV)))   # [D, R]
    e2 = np.ascontiguousarray(np.tile(np.eye(S, dtype=np.float32), (1, 2)))
    i128 = np.eye(128, dtype=np.float32)
    consts = {
        "WqT": np.ascontiguousarray(np.asarray(Wq, np.float32).T),
        "WkT": np.ascontiguousarray(np.asarray(Wk, np.float32).T),
        "WvT": np.ascontiguousarray(np.asarray(Wv, np.float32).T),
        "WoT": np.ascontiguousarray(np.asarray(Wo, np.float32).T),
        "bo": np.ascontiguousarray(np.asarray(bo, np.float32)),
        "peT_rep": peT_rep,
        "pe_nat": pe,
        "E2": e2,
        "I128": i128,
    }
    in_maps = []
    for c in range(N_CORES):
        sl = slice(c * VC, (c + 1) * VC)
        m = dict(consts)
        m["qT"] = np.ascontiguousarray(
            np.asarray(q, np.float32)[sl].reshape(ROWS, D).T)
        m["kT"] = np.ascontiguousarray(
            np.asarray(k, np.float32)[sl].reshape(ROWS, D).T)
        m["vT"] = np.ascontiguousarray(
            np.asarray(v, np.float32)[sl].reshape(ROWS, D).T)
        in_maps.append(m)
    return in_maps


def kernel(q, k, v, Wq, Wk, Wv, Wo, bo, _trace=False):
    from concourse.bass_utils import run_bass_kernel_spmd

    nc = _get_nc()
    in_maps = _host_inputs(q, k, v, Wq, Wk, Wv, Wo, bo)
    res = run_bass_kernel_spmd(nc, in_maps, list(range(N_CORES)), trace=_trace)
    outs = [
        res.results[c]["outT"].reshape(D, VC, S).transpose(1, 2, 0)
        for c in range(N_CORES)
    ]
    full = np.concatenate(outs, axis=0)
    if _trace:
        _CACHE["last_results"] = res
    return full



# revision 37
# speedup vs baseline: 2.1922x; 2.1922x over previous
"""Trainium2 Bass kernel for per-view cross-attention.

Reference computation (per view v of 1024, S=64 samples, D=256):
  qp = q @ Wq.T + pe ; kp = k @ Wk.T + pe ; vp = v @ Wv.T + pe
  attn = softmax(qp @ kp.T / sqrt(D))
  x = gelu(attn @ vp @ Wo.T + bo) + q
Sharding: data-parallel over the 1024 views across 8 cores (128 views each).

Everything lives in "transposed" space [D, rows] (rows = view*64+s) so the
contraction dim D lands on SBUF partitions without on-chip input transposes.
The host pre-transposes shards and post-transposes the output (free: numpy).

v3 design:
  * associativity: (attn @ vp) @ Wo.T == attn @ (vp @ Wo.T), and
    vp @ Wo.T == v @ (Wo @ Wv).T + pe @ Wo.T. The host pre-multiplies
    Wvo = (Wo @ Wv).T and peo = pe @ Wo.T, so the on-chip v-projection
    directly produces vp' = vp @ Wo.T and the attn@v matmul emits the
    final pre-gelu values. The entire Wo projection stage (a third of
    tensor-engine work) and its PSUM evacuations disappear.
  * all matmuls bf16 (f32r pays 4x cycles/row when out free < 256).
    bf16 HBM I/O halves DMA.
  * software pipelining: stage-skewed emission so the in-order PE queue
    always has ready matmuls (proj(t) | scores(t-1) | transpose(t-2) |
    attn@v(t-3)).
  * PSUM rule learned on HW: matmul groups with different tile_position
    ROW offsets must not share a PSUM bank (device crash); same-row
    sequential groups may. attn@v therefore uses one PSUM tile per
    row-half h.
  * engine split per supertile (PE 3.63us is the roofline):
      DVE : qp/kp evac(+pe), vp' pair-1 evac(+pe), softmax reduce +
            reciprocal, preT h=1 evac, residual adds
      ACT : exp, attnT evac, preT h=0 evac, vp' pair-2 evac copy, gelus
      Pool: softmax normalize, attnT partition-half duplicate, vp' pair-2
            pe-add (GPSIMD cannot access PSUM on TRN2)
  * Exp/Gelu sit in different ACT table sets (1283ns/load): gelus grouped
    (GELU_GROUP=8); after the last exp (it >= n_st) pending gelus flush
    immediately, extending the gelu-table era for free.
"""

import sys
import os

for p in ("/opt/trn_rl_repo",):
    if p not in sys.path and os.path.isdir(p):
        sys.path.insert(0, p)

import numpy as np

V, S, D = 1024, 64, 256
N_CORES = 8
VC = V // N_CORES          # views per core
ROWS = VC * S              # 8192 rows per core
R = 512                    # rows per supertile (8 views)
NST = ROWS // R            # supertiles per core
NV = R // S                # views per supertile
GELU_GROUP = 8
SCALE = 1.0 / np.sqrt(np.float32(D)).astype(np.float32)

_CACHE = {}


def _make_posenc(d_hid, n_samples):
    pos = np.arange(n_samples, dtype=np.float64)[:, None]
    j = np.arange(d_hid)[None, :]
    angle = pos / np.power(10000.0, 2.0 * (j // 2) / d_hid)
    table = np.where(j % 2 == 0, np.sin(angle), np.cos(angle))
    return table.astype(np.float32)  # [S, D]


def _build(rows=ROWS):
    import concourse.bass as bass
    import concourse.mybir as mybir
    import concourse.tile as tile
    from concourse.tile import add_dep_helper
    from concourse import bacc
    from contextlib import ExitStack

    fp32 = mybir.dt.float32
    bf16 = mybir.dt.bfloat16
    AF = mybir.ActivationFunctionType
    ALU = mybir.AluOpType
    n_st = rows // R

    nc = bacc.Bacc(None, target_bir_lowering=False)

    qT_d = nc.dram_tensor("qT", [D, rows], bf16, kind="ExternalInput")
    kvT_d = nc.dram_tensor("kvT", [2 * D, rows], bf16, kind="ExternalInput")
    wq_d = nc.dram_tensor("WqT", [D, D], bf16, kind="ExternalInput")
    wk_d = nc.dram_tensor("WkT", [D, D], bf16, kind="ExternalInput")
    wvo_d = nc.dram_tensor("Wvo", [D, D], bf16, kind="ExternalInput")
    bo_d = nc.dram_tensor("bo", [D], fp32, kind="ExternalInput")
    pet_d = nc.dram_tensor("peT_rep", [D, R], fp32, kind="ExternalInput")
    pen_d = nc.dram_tensor("peo_nat", [128, D], fp32, kind="ExternalInput")
    id_d = nc.dram_tensor("I128", [128, 128], bf16, kind="ExternalInput")
    out_d = nc.dram_tensor("outT", [D, rows], bf16, kind="ExternalOutput")

    def r3(ap):  # [kc*128, X] dram -> [128, kc, X] partition view
        return ap.rearrange("(kc p) r -> p kc r", p=128)

    with tile.TileContext(nc) as tc, ExitStack() as ctx:
        const = ctx.enter_context(tc.tile_pool(name="const", bufs=1))
        ldq = ctx.enter_context(tc.tile_pool(name="ldq", bufs=GELU_GROUP + 9))
        ld = ctx.enter_context(tc.tile_pool(name="ld", bufs=3))
        proj = ctx.enter_context(tc.tile_pool(name="proj", bufs=3))
        sm = ctx.enter_context(tc.tile_pool(name="sm", bufs=3))
        vpp = ctx.enter_context(tc.tile_pool(name="vpp", bufs=5))
        stg = ctx.enter_context(tc.tile_pool(name="stg", bufs=GELU_GROUP + 4))
        psP = ctx.enter_context(tc.tile_pool(name="psP", bufs=2, space="PSUM"))
        psSV = ctx.enter_context(tc.tile_pool(name="psSV", bufs=2, space="PSUM"))
        psT = ctx.enter_context(tc.tile_pool(name="psT", bufs=2, space="PSUM"))
        psO = ctx.enter_context(tc.tile_pool(name="psO", bufs=2, space="PSUM"))

        wq = const.tile([128, 2, D], bf16)
        wk = const.tile([128, 2, D], bf16)
        wvo = const.tile([128, 2, D], bf16)
        pet = const.tile([128, 2, R], fp32)
        pen = const.tile([128, D], fp32)
        i128 = const.tile([128, 128], bf16)
        bo_sb = const.tile([128, 2], fp32)

        def load_consts_early():
            nc.sync.dma_start(wq, r3(wq_d[:]))

        def load_consts_mid():
            nc.sync.dma_start(pet, r3(pet_d[:]))
            nc.sync.dma_start(wk, r3(wk_d[:]))

        def load_consts_late():
            nc.sync.dma_start(wvo, r3(wvo_d[:]))
            nc.sync.dma_start(pen, pen_d[:])
            nc.sync.dma_start(i128, id_d[:])
            nc.sync.dma_start(bo_sb, bo_d.rearrange("(kc p) -> p kc", p=128))

        qts = [None] * n_st      # per-st q tiles (residual source)
        qpTs = [None] * n_st
        kpTs = [None] * n_st
        kvts = [None] * n_st
        vps = [None] * n_st
        attns = [None] * n_st
        attnTs = [None] * n_st
        pres = [None] * n_st
        gpend = []               # sts whose preT is staged, gelu not yet
        rpend = []               # sts whose gelu is done, residual+store not
        last_gelu = None
        last_exp = None

        def load(t):
            rs = slice(t * R, (t + 1) * R)
            qts[t] = ldq.tile([128, 2, R], bf16, tag="qt", name="qt")
            kvts[t] = ld.tile([128, 4, R], bf16, tag="kvt", name="kvt")
            nc.sync.dma_start(qts[t], r3(qT_d[:])[:, :, rs])
            nc.sync.dma_start(kvts[t], kvT_d.rearrange(
                "(kc p) r -> p kc r", p=128)[:, :, rs])

        def proj_one(t, w_sb, x_sb, o_sb):
            for mc in range(2):
                ps = psP.tile([128, R], fp32, tag="psP", name="ps_proj")
                for kc in range(2):
                    nc.tensor.matmul(
                        ps,
                        w_sb[:, kc, mc * 128:(mc + 1) * 128],
                        x_sb[:, kc, :],
                        start=(kc == 0),
                        stop=(kc == 1),
                    )
                nc.vector.tensor_add(
                    out=o_sb[:, mc, :], in0=ps, in1=pet[:, mc, :]
                )

        def proj_q(t):
            qpTs[t] = proj.tile([128, 2, R], bf16, tag="qpT", name="qpT")
            proj_one(t, wq, qts[t], qpTs[t])

        def proj_k(t):
            kpTs[t] = proj.tile([128, 2, R], bf16, tag="kpT", name="kpT")
            proj_one(t, wk, kvts[t], kpTs[t])

        def proj_v(t):
            # vp' = v @ Wvo + peo, natural [row, dout2] layout (vT slices as
            # stationary); one PSUM tile per pair of row-groups (same-row
            # sequential accumulation groups may share a bank)
            vps[t] = vpp.tile([128, 4, D], bf16, tag="vp", name="vp")
            vp2raw = sm.tile([128, 2, D], fp32, tag="vp2raw", name="vp2raw")
            for j in range(2):
                psv = psSV.tile([128, 2, D], fp32, tag="psSV", name="ps_vp")
                for gg in range(2):
                    g = 2 * j + gg
                    for kc in range(2):
                        nc.tensor.matmul(
                            psv[:, gg, :],
                            kvts[t][:, 2 + kc, g * 128:(g + 1) * 128],
                            wvo[:, kc, :],
                            start=(kc == 0),
                            stop=(kc == 1),
                        )
                if j == 0:
                    # DVE: evacuate fused with peo add
                    nc.vector.tensor_add(
                        out=vps[t][:, 0:2, :], in0=psv,
                        in1=pen[:, None, :].to_broadcast((128, 2, D)),
                    )
                else:
                    # ACT copy out of PSUM, Pool adds peo SBUF-side
                    nc.scalar.copy(out=vp2raw, in_=psv)
                    nc.gpsimd.tensor_add(
                        out=vps[t][:, 2:4, :], in0=vp2raw,
                        in1=pen[:, None, :].to_broadcast((128, 2, D)),
                    )

        def attn_stage(t):
            nonlocal last_exp
            # scores: per view [64,64], packed [128(2 views), 4, 64]
            scps = psSV.tile([128, 4, S], fp32, tag="psSV", name="ps_scores")
            for v in range(NV):
                g, h = v // 2, v % 2
                for dc in range(2):
                    nc.tensor.matmul(
                        scps[h * 64:(h + 1) * 64, g, :],
                        qpTs[t][:, dc, v * S:(v + 1) * S],
                        kpTs[t][:, dc, v * S:(v + 1) * S],
                        start=(dc == 0),
                        stop=(dc == 1),
                        tile_position=(0, h * 64),
                    )
            # softmax along free axis (no max-subtraction: |s/16| < ~13)
            attns[t] = sm.tile([128, 4, S], bf16, tag="attn", name="attn")
            _e = nc.scalar.activation(attns[t], scps, AF.Exp, scale=float(SCALE))
            if last_gelu is not None:
                add_dep_helper(_e.ins, last_gelu, sync=False,
                               reason="act-table grouping")
            last_exp = _e.ins
            sums = sm.tile([128, 4], fp32, tag="sums", name="sums")
            nc.vector.tensor_reduce(
                out=sums, in_=attns[t], axis=mybir.AxisListType.X, op=ALU.add)
            rec = sm.tile([128, 4], fp32, tag="rec", name="rec")
            nc.vector.reciprocal(rec, sums)
            nc.gpsimd.tensor_tensor(
                attns[t], attns[t],
                rec[:, :, None].to_broadcast((128, 4, S)), ALU.mult,
            )

        def tr_stage(t, tail=False):
            # transpose attn packs into partitions 0-63; the 64-127 duplicate
            # (needed by the h=1 attn@v blocks) is a Pool SBUF copy, not 4
            # more PE matmuls
            atps = psT.tile([64, 4, 128], fp32, tag="psT", name="ps_attnT")
            for g in range(4):
                nc.tensor.matmul(
                    atps[:, g, :],
                    attns[t][:, g, :],
                    i128,
                    start=True,
                    stop=True,
                )
            attnTs[t] = sm.tile([128, 4, 128], bf16, tag="attnT", name="attnT")
            if tail:
                nc.vector.tensor_copy(attnTs[t][0:64, :, :], atps)
            else:
                nc.scalar.copy(out=attnTs[t][0:64, :, :], in_=atps)
            nc.gpsimd.tensor_copy(attnTs[t][64:128, :, :], attnTs[t][0:64, :, :])

        def av_stage(t, tail=False):
            # attn @ vp' -> preT[dout2, row] == the pre-gelu values, packed
            # (g, h, s) per c-chunk. One PSUM tile per row-half h (different
            # tile_position rows must not share a bank); within a tile the
            # c/g groups are same-row sequential.
            pres[t] = stg.tile([128, 2, R], bf16, tag="pre", name="pre")
            for h in range(2):
                pso = psO.tile([128, 2, 4, S], fp32, tag="psO", name="ps_av")
                for c in range(2):
                    for g in range(4):
                        nc.tensor.matmul(
                            pso[:, c, g, :],
                            vps[t][h * 64:(h + 1) * 64, g, c * 128:(c + 1) * 128],
                            attnTs[t][h * 64:(h + 1) * 64, g, h * 64:(h + 1) * 64],
                            start=True,
                            stop=True,
                            tile_position=(h * 64, 0),
                        )
                o_ap = pres[t].rearrange(
                    "p c (g two s) -> p c g two s", two=2, s=S
                )[:, :, :, h, :]
                # ACT carries most preT evacs (DVE is pinned by the qp/kp
                # pe-adds); every 4th supertile h=1 goes to DVE to balance
                on_act = not (h == 1 and t % 4 == 3)
                if on_act and not tail:
                    nc.scalar.copy(out=o_ap, in_=pso)
                else:
                    nc.vector.tensor_copy(o_ap, pso)
            gpend.append(t)

        def gelu_flush():
            nonlocal last_gelu
            for t in gpend:
                outsb = proj.tile([128, 2, R], bf16, tag="outsb", name="outsb",
                                  bufs=GELU_GROUP + 2)
                for c in range(2):
                    _g = nc.scalar.activation(
                        out=outsb[:, c, :], in_=pres[t][:, c, :], func=AF.Gelu,
                        bias=bo_sb[:, c:c + 1], scale=1.0,
                    )
                    if last_exp is not None:
                        add_dep_helper(_g.ins, last_exp, sync=False,
                                       reason="act-table grouping")
                    last_gelu = _g.ins
                rpend.append((t, outsb))
            gpend.clear()

        def resid_store(n, tail=False):
            for _ in range(min(n, len(rpend))):
                t, outsb = rpend.pop(0)
                # Pool mid-stream (emitted after the chain-critical Pool
                # work); DVE at the tail where Pool's slowness would linger
                eng = nc.vector if tail else nc.gpsimd
                eng.tensor_add(out=outsb, in0=outsb, in1=qts[t])
                nc.sync.dma_start(
                    r3(out_d[:])[:, :, t * R:(t + 1) * R], outsb
                )

        load_consts_early()
        load(0)
        load_consts_mid()
        for it in range(n_st + 4):
            if it + 1 < n_st:
                load(it + 1)
            if it == 0:
                load_consts_late()
            if it < n_st:
                proj_q(it)
            if 0 <= it - 1 < n_st:
                attn_stage(it - 1)
            if it < n_st:
                proj_k(it)
            if 0 <= it - 2 < n_st:
                tr_stage(it - 2, tail=it >= n_st)
            if it < n_st:
                proj_v(it)
            if 0 <= it - 3 < n_st:
                av_stage(it - 3, tail=it >= n_st)
            # the last exp is emitted at it == n_st (earlier in this
            # iteration), so late gelus extend the gelu-table era for free
            if len(gpend) >= GELU_GROUP or (it >= n_st and gpend):
                gelu_flush()
            resid_store(1 if it <= n_st else 99, tail=it > n_st)

    nc.finalize()
    return nc


def _get_nc():
    if "nc" not in _CACHE:
        _CACHE["nc"] = _build()
    return _CACHE["nc"]


def _host_inputs(q, k, v, Wq, Wk, Wv, Wo, bo):
    import ml_dtypes

    bf = ml_dtypes.bfloat16
    pe = _make_posenc(D, S)                      # [S, D]
    peT_rep = np.ascontiguousarray(np.tile(pe.T, (1, NV)))   # [D, R]
    Wof = np.asarray(Wo, np.float64)
    Wvo = (Wof @ np.asarray(Wv, np.float64)).T.astype(np.float32)  # [D, D]
    peo = (pe.astype(np.float64) @ Wof.T).astype(np.float32)       # [S, D]
    peo_nat = np.ascontiguousarray(peo[np.arange(128) % 64])       # [128, D]
    i128 = np.eye(128, dtype=bf)
    consts = {
        "WqT": np.ascontiguousarray(np.asarray(Wq, np.float32).T.astype(bf)),
        "WkT": np.ascontiguousarray(np.asarray(Wk, np.float32).T.astype(bf)),
        "Wvo": np.ascontiguousarray(Wvo.astype(bf)),
        "bo": np.ascontiguousarray(np.asarray(bo, np.float32)),
        "peT_rep": peT_rep,
        "peo_nat": peo_nat,
        "I128": i128,
    }
    qf = np.asarray(q, np.float32)
    kf = np.asarray(k, np.float32)
    vf = np.asarray(v, np.float32)
    in_maps = []
    for c in range(N_CORES):
        sl = slice(c * VC, (c + 1) * VC)
        m = dict(consts)
        m["qT"] = np.ascontiguousarray(qf[sl].reshape(ROWS, D).T.astype(bf))
        kT = kf[sl].reshape(ROWS, D).T.astype(bf)
        vT = vf[sl].reshape(ROWS, D).T.astype(bf)
        m["kvT"] = np.ascontiguousarray(np.concatenate([kT, vT], axis=0))
        in_maps.append(m)
    return in_maps


def kernel(q, k, v, Wq, Wk, Wv, Wo, bo, _trace=False):
    from concourse.bass_utils import run_bass_kernel_spmd

    nc = _get_nc()
    in_maps = _host_inputs(q, k, v, Wq, Wk, Wv, Wo, bo)
    res = run_bass_kernel_spmd(nc, in_maps, list(range(N_CORES)), trace=_trace)
    outs = [
        np.asarray(res.results[c]["outT"], dtype=np.float32)
        .reshape(D, VC, S).transpose(1, 2, 0)
        for c in range(N_CORES)
    ]
    full = np.concatenate(outs, axis=0)
    if _trace:
        _CACHE["last_results"] = res
    return full


# revision 55
# speedup vs baseline: 2.3858x; 1.0883x over previous
"""Trainium2 Bass kernel for per-view cross-attention.

Reference computation (per view v of 1024, S=64 samples, D=256):
  qp = q @ Wq.T + pe ; kp = k @ Wk.T + pe ; vp = v @ Wv.T + pe
  attn = softmax(qp @ kp.T / sqrt(D))
  x = gelu(attn @ vp @ Wo.T + bo) + q
Sharding: data-parallel over the 1024 views across 8 cores (128 views each).

Everything lives in "transposed" space [D, rows] (rows = view*64+s) so the
contraction dim D lands on SBUF partitions without on-chip input transposes.
The host pre-transposes shards and post-transposes the output (free: numpy).

v3 design:
  * associativity: (attn @ vp) @ Wo.T == attn @ (vp @ Wo.T), and
    vp @ Wo.T == v @ (Wo @ Wv).T + pe @ Wo.T. The host pre-multiplies
    Wvo = (Wo @ Wv).T and peo = pe @ Wo.T, so the on-chip v-projection
    directly produces vp' = vp @ Wo.T and the attn@v matmul emits the
    final pre-gelu values. The entire Wo projection stage (a third of
    tensor-engine work) and its PSUM evacuations disappear.
  * all matmuls bf16 (f32r pays 4x cycles/row when out free < 256).
    bf16 HBM I/O halves DMA.
  * software pipelining: stage-skewed emission so the in-order PE queue
    always has ready matmuls (proj(t) | scores(t-1) | transpose(t-2) |
    attn@v(t-3)).
  * PSUM rule learned on HW: matmul groups with different tile_position
    ROW offsets must not share a PSUM bank (device crash); same-row
    sequential groups may. attn@v therefore uses one PSUM tile per
    row-half h.
  * engine split per supertile (PE 3.63us is the roofline):
      DVE : qp/kp evac(+pe), vp' pair-1 evac(+pe), softmax reduce +
            reciprocal, preT h=1 evac, residual adds
      ACT : exp, attnT evac, preT h=0 evac, vp' pair-2 evac copy, gelus
      Pool: softmax normalize, attnT partition-half duplicate, vp' pair-2
            pe-add (GPSIMD cannot access PSUM on TRN2)
  * Exp/Gelu sit in different ACT table sets (1283ns/load): gelus grouped
    (GELU_GROUP=8); after the last exp (it >= n_st) pending gelus flush
    immediately, extending the gelu-table era for free.
"""

import sys
import os

for p in ("/opt/trn_rl_repo",):
    if p not in sys.path and os.path.isdir(p):
        sys.path.insert(0, p)

import numpy as np

V, S, D = 1024, 64, 256
N_CORES = 8
VC = V // N_CORES          # views per core
ROWS = VC * S              # 8192 rows per core
R = 512                    # rows per supertile (8 views)
NST = ROWS // R            # supertiles per core
NV = R // S                # views per supertile
GELU_GROUP = 8
SCALE = 1.0 / np.sqrt(np.float32(D)).astype(np.float32)

_CACHE = {}


def _make_posenc(d_hid, n_samples):
    pos = np.arange(n_samples, dtype=np.float64)[:, None]
    j = np.arange(d_hid)[None, :]
    angle = pos / np.power(10000.0, 2.0 * (j // 2) / d_hid)
    table = np.where(j % 2 == 0, np.sin(angle), np.cos(angle))
    return table.astype(np.float32)  # [S, D]


def _build(rows=ROWS):
    import concourse.bass as bass
    import concourse.mybir as mybir
    import concourse.tile as tile
    from concourse.tile import add_dep_helper
    from concourse import bacc
    from contextlib import ExitStack

    fp32 = mybir.dt.float32
    bf16 = mybir.dt.bfloat16
    AF = mybir.ActivationFunctionType
    ALU = mybir.AluOpType
    n_st = rows // R

    nc = bacc.Bacc(None, target_bir_lowering=False)

    qT_d = nc.dram_tensor("qT", [D, rows], bf16, kind="ExternalInput")
    kvT_d = nc.dram_tensor("kvT", [2 * D, rows], bf16, kind="ExternalInput")
    wq_d = nc.dram_tensor("WqT", [D, D], bf16, kind="ExternalInput")
    wk_d = nc.dram_tensor("WkT", [D, D], bf16, kind="ExternalInput")
    wvo_d = nc.dram_tensor("Wvo", [D, D], bf16, kind="ExternalInput")
    bo_d = nc.dram_tensor("bo", [D], fp32, kind="ExternalInput")
    pet_d = nc.dram_tensor("peT_rep", [D, R], fp32, kind="ExternalInput")
    pen_d = nc.dram_tensor("peo_nat", [128, D], fp32, kind="ExternalInput")
    id_d = nc.dram_tensor("I128", [128, 128], bf16, kind="ExternalInput")
    out_d = nc.dram_tensor("outT", [D, rows], bf16, kind="ExternalOutput")

    def r3(ap):  # [kc*128, X] dram -> [128, kc, X] partition view
        return ap.rearrange("(kc p) r -> p kc r", p=128)

    with tile.TileContext(nc) as tc, ExitStack() as ctx:
        const = ctx.enter_context(tc.tile_pool(name="const", bufs=1))
        ldq = ctx.enter_context(tc.tile_pool(name="ldq", bufs=GELU_GROUP + 9))
        ld = ctx.enter_context(tc.tile_pool(name="ld", bufs=4))
        proj = ctx.enter_context(tc.tile_pool(name="proj", bufs=4))
        sm = ctx.enter_context(tc.tile_pool(name="sm", bufs=5))
        vpp = ctx.enter_context(tc.tile_pool(name="vpp", bufs=7))
        stg = ctx.enter_context(tc.tile_pool(name="stg", bufs=GELU_GROUP + 4))
        psP = ctx.enter_context(tc.tile_pool(name="psP", bufs=2, space="PSUM"))
        psSV = ctx.enter_context(tc.tile_pool(name="psSV", bufs=2, space="PSUM"))
        psT = ctx.enter_context(tc.tile_pool(name="psT", bufs=2, space="PSUM"))
        psO = ctx.enter_context(tc.tile_pool(name="psO", bufs=2, space="PSUM"))

        wq = const.tile([128, 2, D], bf16)
        wk = const.tile([128, 2, D], bf16)
        wvo = const.tile([128, 2, D], bf16)
        pet = const.tile([128, 2, R], fp32)
        pen = const.tile([128, D], fp32)
        i128 = const.tile([128, 128], bf16)
        bo_sb = const.tile([128, 2], fp32)

        def load_consts_early():
            nc.sync.dma_start(wq, r3(wq_d[:]))

        def load_consts_mid():
            nc.sync.dma_start(pet, r3(pet_d[:]))
            nc.sync.dma_start(wk, r3(wk_d[:]))

        def load_consts_late():
            nc.sync.dma_start(wvo, r3(wvo_d[:]))
            nc.sync.dma_start(pen, pen_d[:])
            nc.sync.dma_start(i128, id_d[:])
            nc.sync.dma_start(bo_sb, bo_d.rearrange("(kc p) -> p kc", p=128))

        qts = [None] * n_st      # per-st q tiles (residual source)
        qpTs = [None] * n_st
        kpTs = [None] * n_st
        kvts = [None] * n_st
        vps = [None] * n_st
        attns = [None] * n_st
        attnTs = [None] * n_st
        pres = [None] * n_st
        gpend = []               # sts whose preT is staged, gelu not yet
        rpend = []               # sts whose gelu is done, residual+store not
        last_gelu = None
        last_exp = None

        def load(t):
            rs = slice(t * R, (t + 1) * R)
            qts[t] = ldq.tile([128, 2, R], bf16, tag="qt", name="qt")
            kvts[t] = ld.tile([128, 4, R], bf16, tag="kvt", name="kvt")
            nc.sync.dma_start(qts[t], r3(qT_d[:])[:, :, rs])
            nc.sync.dma_start(kvts[t], kvT_d.rearrange(
                "(kc p) r -> p kc r", p=128)[:, :, rs])

        def proj_one(t, w_sb, x_sb, o_sb):
            for mc in range(2):
                ps = psP.tile([128, R], fp32, tag="psP", name="ps_proj")
                for kc in range(2):
                    nc.tensor.matmul(
                        ps,
                        w_sb[:, kc, mc * 128:(mc + 1) * 128],
                        x_sb[:, kc, :],
                        start=(kc == 0),
                        stop=(kc == 1),
                    )
                nc.vector.tensor_add(
                    out=o_sb[:, mc, :], in0=ps, in1=pet[:, mc, :]
                )

        def proj_q(t):
            qpTs[t] = proj.tile([128, 2, R], bf16, tag="qpT", name="qpT")
            proj_one(t, wq, qts[t], qpTs[t])

        def proj_k(t):
            kpTs[t] = proj.tile([128, 2, R], bf16, tag="kpT", name="kpT")
            proj_one(t, wk, kvts[t], kpTs[t])

        def proj_v(t):
            # vp' = v @ Wvo + peo, natural [row, dout2] layout (vT slices as
            # stationary); one PSUM tile per pair of row-groups (same-row
            # sequential accumulation groups may share a bank)
            vps[t] = vpp.tile([128, 4, D], bf16, tag="vp", name="vp")
            vp2raw = sm.tile([128, 4, D], fp32, tag="vp2raw", name="vp2raw")
            for j in range(2):
                psv = psSV.tile([128, 2, D], fp32, tag="psSV", name="ps_vp")
                for gg in range(2):
                    g = 2 * j + gg
                    for kc in range(2):
                        nc.tensor.matmul(
                            psv[:, gg, :],
                            kvts[t][:, 2 + kc, g * 128:(g + 1) * 128],
                            wvo[:, kc, :],
                            start=(kc == 0),
                            stop=(kc == 1),
                        )
                if j == 0 and t % 2 == 0:
                    # DVE: evacuate fused with peo add
                    nc.vector.tensor_add(
                        out=vps[t][:, 0:2, :], in0=psv,
                        in1=pen[:, None, :].to_broadcast((128, 2, D)),
                    )
                elif j == 0:
                    nc.scalar.copy(out=vp2raw[:, 0:2, :], in_=psv)
                    nc.gpsimd.tensor_add(
                        out=vps[t][:, 0:2, :], in0=vp2raw[:, 0:2, :],
                        in1=pen[:, None, :].to_broadcast((128, 2, D)),
                    )
                else:
                    # ACT copy out of PSUM, Pool adds peo SBUF-side
                    nc.scalar.copy(out=vp2raw[:, 2:4, :], in_=psv)
                    nc.gpsimd.tensor_add(
                        out=vps[t][:, 2:4, :], in0=vp2raw[:, 2:4, :],
                        in1=pen[:, None, :].to_broadcast((128, 2, D)),
                    )

        def attn_stage(t):
            nonlocal last_exp
            # scores: per view [64,64], packed [128(2 views), 4, 64]
            scps = psSV.tile([128, 4, S], fp32, tag="psSV", name="ps_scores")
            for v in range(NV):
                g, h = v // 2, v % 2
                for dc in range(2):
                    nc.tensor.matmul(
                        scps[h * 64:(h + 1) * 64, g, :],
                        qpTs[t][:, dc, v * S:(v + 1) * S],
                        kpTs[t][:, dc, v * S:(v + 1) * S],
                        start=(dc == 0),
                        stop=(dc == 1),
                        tile_position=(0, h * 64),
                    )
            # softmax along free axis (no max-subtraction: |s/16| < ~13)
            attns[t] = sm.tile([128, 4, S], bf16, tag="attn", name="attn")
            _e = nc.scalar.activation(attns[t], scps, AF.Exp, scale=float(SCALE))
            if last_gelu is not None:
                add_dep_helper(_e.ins, last_gelu, sync=False,
                               reason="act-table grouping")
            last_exp = _e.ins
            sums = sm.tile([128, 4], fp32, tag="sums", name="sums")
            nc.vector.tensor_reduce(
                out=sums, in_=attns[t], axis=mybir.AxisListType.X, op=ALU.add)
            rec = sm.tile([128, 4], fp32, tag="rec", name="rec")
            nc.vector.reciprocal(rec, sums)
            nc.gpsimd.tensor_tensor(
                attns[t], attns[t],
                rec[:, :, None].to_broadcast((128, 4, S)), ALU.mult,
            )

        def tr_stage(t, tail=False):
            # transpose attn packs into partitions 0-63; the 64-127 duplicate
            # (needed by the h=1 attn@v blocks) is a Pool SBUF copy, not 4
            # more PE matmuls
            atps = psT.tile([64, 4, 128], fp32, tag="psT", name="ps_attnT")
            for g in range(4):
                nc.tensor.matmul(
                    atps[:, g, :],
                    attns[t][:, g, :],
                    i128,
                    start=True,
                    stop=True,
                )
            attnTs[t] = sm.tile([128, 4, 128], bf16, tag="attnT", name="attnT")
            if tail:
                nc.vector.tensor_copy(attnTs[t][0:64, :, :], atps)
            else:
                nc.scalar.copy(out=attnTs[t][0:64, :, :], in_=atps)
            nc.gpsimd.tensor_copy(attnTs[t][64:128, :, :], attnTs[t][0:64, :, :])

        def av_stage(t, tail=False):
            # attn @ vp' -> preT[dout2, row] == the pre-gelu values, packed
            # (g, h, s) per c-chunk. One PSUM tile per row-half h (different
            # tile_position rows must not share a bank); within a tile the
            # c/g groups are same-row sequential.
            pres[t] = stg.tile([128, 2, R], bf16, tag="pre", name="pre")
            for h in range(2):
                pso = psO.tile([128, 2, 4, S], fp32, tag="psO", name="ps_av")
                for c in range(2):
                    for g in range(4):
                        nc.tensor.matmul(
                            pso[:, c, g, :],
                            vps[t][h * 64:(h + 1) * 64, g, c * 128:(c + 1) * 128],
                            attnTs[t][h * 64:(h + 1) * 64, g, h * 64:(h + 1) * 64],
                            start=True,
                            stop=True,
                            tile_position=(h * 64, 0),
                        )
                o_ap = pres[t].rearrange(
                    "p c (g two s) -> p c g two s", two=2, s=S
                )[:, :, :, h, :]
                # ACT carries the preT evacs (DVE is pinned by the qp/kp
                # pe-adds); the tail switches to DVE behind the gelu burst
                if not tail:
                    nc.scalar.copy(out=o_ap, in_=pso)
                else:
                    nc.vector.tensor_copy(o_ap, pso)
            gpend.append(t)

        def gelu_flush():
            nonlocal last_gelu
            for t in gpend:
                outsb = proj.tile([128, 2, R], bf16, tag="outsb", name="outsb",
                                  bufs=GELU_GROUP + 2)
                for c in range(2):
                    _g = nc.scalar.activation(
                        out=outsb[:, c, :], in_=pres[t][:, c, :], func=AF.Gelu,
                        bias=bo_sb[:, c:c + 1], scale=1.0,
                    )
                    if last_exp is not None:
                        add_dep_helper(_g.ins, last_exp, sync=False,
                                       reason="act-table grouping")
                    last_gelu = _g.ins
                rpend.append((t, outsb))
            gpend.clear()

        def resid_store(n, tail=False):
            for _ in range(min(n, len(rpend))):
                t, outsb = rpend.pop(0)
                # Pool mid-stream (emitted after the chain-critical Pool
                # work); DVE at the tail where Pool's slowness would linger
                eng = nc.vector if tail else nc.gpsimd
                eng.tensor_add(out=outsb, in0=outsb, in1=qts[t])
                nc.sync.dma_start(
                    r3(out_d[:])[:, :, t * R:(t + 1) * R], outsb
                )

        load_consts_early()
        load(0)
        load_consts_mid()
        load(1)
        flush_now = False
        for it in range(n_st + 5):
            if it + 2 < n_st:
                load(it + 2)
            if it == 0:
                load_consts_late()
            if it < n_st:
                proj_q(it)
            if 0 <= it - 1 < n_st:
                attn_stage(it - 1)
            # deferred gelu flush: emitted right after this iteration's exp
            # so the softmax chain never queues behind the gelu burst
            if flush_now:
                gelu_flush()
                flush_now = False
            if it < n_st:
                proj_k(it)
            if 0 <= it - 3 < n_st:
                tr_stage(it - 3, tail=it >= n_st)
            if it < n_st:
                proj_v(it)
            if 0 <= it - 4 < n_st:
                av_stage(it - 4, tail=it >= n_st)
            # the last exp is emitted at it == n_st, so late gelus extend
            # the gelu-table era for free
            if it >= n_st and gpend:
                gelu_flush()
            elif len(gpend) >= GELU_GROUP:
                flush_now = True
            resid_store(1 if it <= n_st else 99, tail=it > n_st)

    nc.finalize()
    return nc


def _get_nc():
    if "nc" not in _CACHE:
        _CACHE["nc"] = _build()
    return _CACHE["nc"]


def _host_inputs(q, k, v, Wq, Wk, Wv, Wo, bo):
    import ml_dtypes

    bf = ml_dtypes.bfloat16
    pe = _make_posenc(D, S)                      # [S, D]
    peT_rep = np.ascontiguousarray(np.tile(pe.T, (1, NV)))   # [D, R]
    Wof = np.asarray(Wo, np.float64)
    Wvo = (Wof @ np.asarray(Wv, np.float64)).T.astype(np.float32)  # [D, D]
    peo = (pe.astype(np.float64) @ Wof.T).astype(np.float32)       # [S, D]
    peo_nat = np.ascontiguousarray(peo[np.arange(128) % 64])       # [128, D]
    i128 = np.eye(128, dtype=bf)
    consts = {
        "WqT": np.ascontiguousarray(np.asarray(Wq, np.float32).T.astype(bf)),
        "WkT": np.ascontiguousarray(np.asarray(Wk, np.float32).T.astype(bf)),
        "Wvo": np.ascontiguousarray(Wvo.astype(bf)),
        "bo": np.ascontiguousarray(np.asarray(bo, np.float32)),
        "peT_rep": peT_rep,
        "peo_nat": peo_nat,
        "I128": i128,
    }
    qf = np.asarray(q, np.float32)
    kf = np.asarray(k, np.float32)
    vf = np.asarray(v, np.float32)
    in_maps = []
    for c in range(N_CORES):
        sl = slice(c * VC, (c + 1) * VC)
        m = dict(consts)
        m["qT"] = np.ascontiguousarray(qf[sl].reshape(ROWS, D).T.astype(bf))
        kT = kf[sl].reshape(ROWS, D).T.astype(bf)
        vT = vf[sl].reshape(ROWS, D).T.astype(bf)
        m["kvT"] = np.ascontiguousarray(np.concatenate([kT, vT], axis=0))
        in_maps.append(m)
    return in_maps


def kernel(q, k, v, Wq, Wk, Wv, Wo, bo, _trace=False):
    from concourse.bass_utils import run_bass_kernel_spmd

    nc = _get_nc()
    in_maps = _host_inputs(q, k, v, Wq, Wk, Wv, Wo, bo)
    res = run_bass_kernel_spmd(nc, in_maps, list(range(N_CORES)), trace=_trace)
    outs = [
        np.asarray(res.results[c]["outT"], dtype=np.float32)
        .reshape(D, VC, S).transpose(1, 2, 0)
        for c in range(N_CORES)
    ]
    full = np.concatenate(outs, axis=0)
    if _trace:
        _CACHE["last_results"] = res
    return full
